# revision 20
# baseline (speedup 1.0000x reference)
"""AR(1) sequence generation kernel for Trainium2 (8 NeuronCores).

x_t = COEFF * x_{t-1} + STD * eps_t  with x_0 given; output [B, T] f32.

Algorithm: the recurrence is a causal convolution x = conv(E, k) with
k[d] = STD * COEFF**d, where E[:,0] = x0/STD and E[:,1:] = eps.  Since
COEFF**128 ~ 4e-13 (far below f32 ulp), a 128-wide output time-block m
only depends on time-blocks m and m-1:

    X[:, m*128:(m+1)*128] = E_m @ M0 + E_{m-1} @ M1

with constant 128x128 Toeplitz matrices M0[j,i] = k[i-j] (i>=j) and
M1[j,i] = k[128+i-j].  This removes the serial scan entirely.

Precision/speed: operands are split on the host into bf16 hi + bf16 lo
(same DMA bytes as f32).  Each logical f32 matmul becomes three bf16
matmuls accumulated in f32 PSUM: hi@hi + hi@lo + lo@hi, giving ~1e-5
relative error (vs ~3e-4 for TF32) at full bf16 TensorEngine rate.

Sharding: pure data parallel over batch (8192 -> 8 x 1024).  Compute
uses a transposed layout (time on partitions, batch on the free dim) so
the constant matrices are the stationary operand and the moving operand
streams 512 batch columns per matmul; the host transposes input/output.
"""

import numpy as np

COEFF = 0.8
STD = 0.1
B, T = 8192, 4096
NCORES = 8
BL = B // NCORES  # 1024 batch rows per core
P = 128           # partition dim / time block
NBLK = T // P     # 32 time blocks
NCHUNK = BL // 512  # 2 batch chunks of 512

# "bf16x2p" (hi/lo split + packed DMA layout), "bf16x2" (hi/lo split),
# or "f32" (exact, 4x slower PE)
MM_DTYPE = "fp16p"
NSB = NBLK // 2  # superblocks of 2 time blocks (packed layout)

_CACHE = {}


def _toeplitz():
    j = np.arange(P, dtype=np.float64)[:, None]
    i = np.arange(P, dtype=np.float64)[None, :]
    m0 = np.where(i >= j, STD * COEFF ** (i - j), 0.0).astype(np.float32)
    m1 = (STD * COEFF ** (P + i - j)).astype(np.float32)
    return m0, m1


def _split_bf16(a):
    import ml_dtypes

    bf16 = ml_dtypes.bfloat16
    hi = a.astype(bf16)
    lo = (a - hi.astype(np.float32)).astype(bf16)
    return hi, lo


def _build_nc_f32():
    import concourse.bacc as bacc
    import concourse.tile as tile
    import concourse.mybir as mybir

    f32 = mybir.dt.float32
    nc = bacc.Bacc("TRN2", target_bir_lowering=False, debug=False)
    ein = nc.dram_tensor("ein", [T, BL], f32, kind="ExternalInput")
    m0 = nc.dram_tensor("m0", [P, P], f32, kind="ExternalInput")
    m1 = nc.dram_tensor("m1", [P, P], f32, kind="ExternalInput")
    out = nc.dram_tensor("out", [T, BL], f32, kind="ExternalOutput")

    with tile.TileContext(nc) as tc:
        with (
            tc.tile_pool(name="consts", bufs=1) as consts,
            tc.tile_pool(name="einp", bufs=4) as einp,
            tc.tile_pool(name="stage", bufs=3) as stage,
            tc.tile_pool(name="psum", bufs=4, space="PSUM") as psump,
        ):
            m0t = consts.tile([P, P], f32, name="m0t")
            nc.sync.dma_start(m0t[:], m0[:, :])
            m1t = consts.tile([P, P], f32, name="m1t")
            nc.sync.dma_start(m1t[:], m1[:, :])

            einv = ein[:, :].rearrange("(n p) b -> n p b", p=P)
            outv = out[:, :].rearrange("(n p) b -> n p b", p=P)

            prev = None
            for m in range(NBLK):
                et = einp.tile([P, BL], f32, name="et")
                nc.sync.dma_start(et[:], einv[m])
                st = stage.tile([P, BL], f32, name="st")
                for c in range(NCHUNK):
                    ps = psump.tile([P, 512], f32, name="ps")
                    sl = slice(c * 512, (c + 1) * 512)
                    nc.tensor.matmul(
                        ps[:], m0t[:], et[:, sl],
                        start=True, stop=(prev is None),
                    )
                    if prev is not None:
                        nc.tensor.matmul(
                            ps[:], m1t[:], prev[:, sl],
                            start=False, stop=True,
                        )
                    nc.vector.tensor_copy(out=st[:, sl], in_=ps[:])
                nc.sync.dma_start(outv[m], st[:])
                prev = et
    nc.compile()
    return nc


def _build_nc_bf16x2():
    import concourse.bacc as bacc
    import concourse.tile as tile
    import concourse.mybir as mybir

    f32 = mybir.dt.float32
    bf16 = mybir.dt.bfloat16
    nc = bacc.Bacc("TRN2", target_bir_lowering=False, debug=False)
    ein_hi = nc.dram_tensor("ein_hi", [T, BL], bf16, kind="ExternalInput")
    ein_lo = nc.dram_tensor("ein_lo", [T, BL], bf16, kind="ExternalInput")
    consts_in = {}
    for name in ("m0h", "m0l", "m1h", "m1l"):
        consts_in[name] = nc.dram_tensor(name, [P, P], bf16, kind="ExternalInput")
    out = nc.dram_tensor("out", [T, BL], f32, kind="ExternalOutput")

    with tile.TileContext(nc) as tc:
        with (
            tc.tile_pool(name="consts", bufs=1) as consts,
            tc.tile_pool(name="einp", bufs=8) as einp,
            tc.tile_pool(name="stage", bufs=3) as stage,
            tc.tile_pool(name="psum", bufs=4, space="PSUM") as psump,
        ):
            ct = {}
            for name in ("m0h", "m0l", "m1h", "m1l"):
                ct[name] = consts.tile([P, P], bf16, name=name + "t")
                nc.sync.dma_start(ct[name][:], consts_in[name][:, :])

            ehv = ein_hi[:, :].rearrange("(n p) b -> n p b", p=P)
            elv = ein_lo[:, :].rearrange("(n p) b -> n p b", p=P)
            outv = out[:, :].rearrange("(n p) b -> n p b", p=P)

            prev = None
            for m in range(NBLK):
                eh = einp.tile([P, BL], bf16, name="eh", tag="eh")
                nc.sync.dma_start(eh[:], ehv[m])
                el = einp.tile([P, BL], bf16, name="el", tag="el")
                nc.sync.dma_start(el[:], elv[m])
                st = stage.tile([P, BL], f32, name="st")
                for c in range(NCHUNK):
                    sl = slice(c * 512, (c + 1) * 512)
                    ps = psump.tile([P, 512], f32, name="ps")
                    # (weights, moving) products accumulated in PSUM;
                    # ordered so equal stationary operands are adjacent.
                    prods = [
                        (ct["m0h"], eh),
                        (ct["m0h"], el),
                        (ct["m0l"], eh),
                    ]
                    if prev is not None:
                        ph, pl = prev
                        prods += [
                            (ct["m1h"], ph),
                            (ct["m1h"], pl),
                            (ct["m1l"], ph),
                        ]
                    last = len(prods) - 1
                    for idx, (w, mv) in enumerate(prods):
                        nc.tensor.matmul(
                            ps[:], w[:], mv[:, sl],
                            start=(idx == 0), stop=(idx == last),
                        )
                    nc.vector.tensor_copy(out=st[:, sl], in_=ps[:])
                nc.sync.dma_start(outv[m], st[:])
                prev = (eh, el)
    nc.compile()
    return nc


def _build_nc_bf16x2p():
    """Packed layout: input DRAM tensor [NSB*128, 4096] bf16 where the
    row (sb*128 + p) holds, contiguously: [hi(sub0) | lo(sub0) | hi(sub1)
    | lo(sub1)] each 1024 wide, with sub-block rows interleaved so every
    partition line is one contiguous 8KB DMA descriptor.  Output DRAM is
    [NSB*128, 2048] f32: row (sb*128+p) = [x^T row 256sb+p | row
    256sb+128+p] (8KB lines); the host unpacks."""
    import concourse.bacc as bacc
    import concourse.tile as tile
    import concourse.mybir as mybir

    f32 = mybir.dt.float32
    bf16 = mybir.dt.bfloat16
    nc = bacc.Bacc("TRN2", target_bir_lowering=False, debug=False)
    einp = nc.dram_tensor("einp", [NSB * P, 4096], bf16, kind="ExternalInput")
    consts_in = {}
    for name in ("m0h", "m0l", "m1h", "m1l"):
        consts_in[name] = nc.dram_tensor(name, [P, P], bf16, kind="ExternalInput")
    outp = nc.dram_tensor("outp", [NSB * P, 2048], f32, kind="ExternalOutput")

    with tile.TileContext(nc) as tc:
        with (
            tc.tile_pool(name="consts", bufs=1) as consts,
            tc.tile_pool(name="ein", bufs=3) as ein_pool,
            tc.tile_pool(name="stage", bufs=3) as stage,
            tc.tile_pool(name="psum", bufs=8, space="PSUM") as psump,
        ):
            ct = {}
            for name in ("m0h", "m0l", "m1h", "m1l"):
                ct[name] = consts.tile([P, P], bf16, name=name + "t")
                nc.gpsimd.dma_start(ct[name][:], consts_in[name][:, :])

            # [NSB, p, sub, hilo, 1024]
            einv = einp[:, :].rearrange(
                "(n p) (s h b) -> n p s h b", p=P, s=2, h=2
            )
            outv = outp[:, :].rearrange("(n p) (s b) -> n p s b", p=P, s=2)

            prev = None  # (tile, sub) holding time block m-1
            for sb in range(NSB):
                et = ein_pool.tile([P, 2, 2, 1024], bf16, name="et")
                nc.sync.dma_start(et[:], einv[sb])
                st = stage.tile([P, 2, 1024], f32, name="st")
                for sub in range(2):
                    eh = et[:, sub, 0, :]
                    el = et[:, sub, 1, :]
                    for c in range(NCHUNK):
                        sl = slice(c * 512, (c + 1) * 512)
                        ps = psump.tile([P, 512], f32, name="ps")
                        prods = [
                            (ct["m0h"], eh),
                            (ct["m0h"], el),
                            (ct["m0l"], eh),
                        ]
                        if prev is not None:
                            pt, psub = prev
                            prods += [
                                (ct["m1h"], pt[:, psub, 0, :]),
                                (ct["m1h"], pt[:, psub, 1, :]),
                                (ct["m1l"], pt[:, psub, 0, :]),
                            ]
                        last = len(prods) - 1
                        for idx, (w, mv) in enumerate(prods):
                            nc.tensor.matmul(
                                ps[:], w[:], mv[:, sl],
                                start=(idx == 0), stop=(idx == last),
                            )
                        nc.vector.tensor_copy(out=st[:, sub, sl], in_=ps[:])
                    prev = (et, sub)
                nc.scalar.dma_start(outv[sb], st[:])
    nc.compile()
    return nc


def _build_nc_fp16p(subs=2, ein_bufs=6, stage_bufs=4):
    """Pure fp16 operands (rel err ~3e-4), packed DMA layout.

    Input DRAM [(NBLK/subs)*P, subs*1024] fp16: row (g*P+p) holds the
    `subs` E^T rows {subs*128*g + 128*s + p} concatenated, so every
    partition line is one contiguous (2KB*subs) DMA descriptor.  Output
    DRAM is packed the same way in f32 (4KB*subs lines); the host
    unpacks.  Two matmul products per output chunk."""
    import concourse.bacc as bacc
    import concourse.tile as tile
    import concourse.mybir as mybir

    ngrp = NBLK // subs
    f32 = mybir.dt.float32
    f16 = mybir.dt.float16
    nc = bacc.Bacc("TRN2", target_bir_lowering=False, debug=False)
    einp = nc.dram_tensor("einp", [ngrp * P, subs * BL], f16, kind="ExternalInput")
    m0_in = nc.dram_tensor("m0", [P, P], f16, kind="ExternalInput")
    m1_in = nc.dram_tensor("m1", [P, P], f16, kind="ExternalInput")
    outp = nc.dram_tensor("outp", [ngrp * P, subs * BL], f32, kind="ExternalOutput")

    with tile.TileContext(nc) as tc:
        with (
            tc.tile_pool(name="consts", bufs=1) as consts,
            tc.tile_pool(name="ein", bufs=ein_bufs) as ein_pool,
            tc.tile_pool(name="stage", bufs=stage_bufs) as stage,
            tc.tile_pool(name="psum", bufs=8, space="PSUM") as psump,
        ):
            m0t = consts.tile([P, P], f16, name="m0t")
            nc.sync.dma_start(m0t[:], m0_in[:, :])
            m1t = consts.tile([P, P], f16, name="m1t")
            nc.sync.dma_start(m1t[:], m1_in[:, :])

            einv = einp[:, :].rearrange("(n p) (s b) -> n p s b", p=P, s=subs)
            outv = outp[:, :].rearrange("(n p) (s b) -> n p s b", p=P, s=subs)

            prev = None  # (tile, sub) holding time block m-1
            for g in range(ngrp):
                et = ein_pool.tile([P, subs, BL], f16, name="et")
                nc.sync.dma_start(et[:], einv[g])
                st = stage.tile([P, subs, BL], f32, name="st")
                for sub in range(subs):
                    for c in range(NCHUNK):
                        sl = slice(c * 512, (c + 1) * 512)
                        ps = psump.tile([P, 512], f32, name="ps")
                        if prev is None:
                            nc.tensor.matmul(
                                ps[:], m0t[:], et[:, sub, sl],
                                start=True, stop=True,
                            )
                        else:
                            pt, psub = prev
                            nc.tensor.matmul(
                                ps[:], m0t[:], et[:, sub, sl],
                                start=True, stop=False,
                            )
                            nc.tensor.matmul(
                                ps[:], m1t[:], pt[:, psub, sl],
                                start=False, stop=True,
                            )
                        if (sub + c) % 2 == 0:
                            nc.vector.tensor_copy(out=st[:, sub, sl], in_=ps[:])
                        else:
                            nc.scalar.copy(out=st[:, sub, sl], in_=ps[:])
                    prev = (et, sub)
                nc.scalar.dma_start(outv[g], st[:])
    nc.compile()
    return nc


def _get_nc(mm_dtype=MM_DTYPE):
    if mm_dtype not in _CACHE:
        if mm_dtype.startswith("fp16p"):
            subs = int(mm_dtype[5:]) if len(mm_dtype) > 5 else 2
            kw = {"ein_bufs": 3, "stage_bufs": 3} if subs >= 8 else {}
            _CACHE[mm_dtype] = _build_nc_fp16p(subs=subs, **kw)
        elif mm_dtype == "bf16x2p":
            _CACHE[mm_dtype] = _build_nc_bf16x2p()
        elif mm_dtype == "bf16x2":
            _CACHE[mm_dtype] = _build_nc_bf16x2()
        elif mm_dtype == "f32":
            _CACHE[mm_dtype] = _build_nc_f32()
        else:
            raise ValueError(mm_dtype)
    return _CACHE[mm_dtype]


def _make_e(x0, eps):
    e = np.empty((B, T), np.float32)
    e[:, 0] = (x0[:, 0].astype(np.float64) / STD).astype(np.float32)
    e[:, 1:] = eps
    return np.ascontiguousarray(e.T)  # [T, B]


def _pack_input(shard):
    """shard: [T, BL] f32 (x^T for one core) -> [NSB*P, 4096] bf16 packed."""
    hi, lo = _split_bf16(shard)
    # [NSB, sub, P, BL] -> [NSB, P, sub, BL]
    hi4 = hi.reshape(NSB, 2, P, BL).transpose(0, 2, 1, 3)
    lo4 = lo.reshape(NSB, 2, P, BL).transpose(0, 2, 1, 3)
    packed = np.empty((NSB, P, 2, 2, BL), hi.dtype)
    packed[:, :, :, 0, :] = hi4
    packed[:, :, :, 1, :] = lo4
    return np.ascontiguousarray(packed.reshape(NSB * P, 4 * BL))


def _unpack_output(arr, subs=2):
    """arr: [(NBLK/subs)*P, subs*BL] f32 -> [BL, T] (batch-major shard)."""
    ngrp = NBLK // subs
    a = arr.reshape(ngrp, P, subs, BL).transpose(0, 2, 1, 3).reshape(T, BL)
    return a.T


def _pack_input_fp16(shard, subs=2):
    """shard: [T, BL] f32 -> [(NBLK/subs)*P, subs*BL] fp16 packed."""
    ngrp = NBLK // subs
    h = shard.astype(np.float16).reshape(ngrp, subs, P, BL).transpose(0, 2, 1, 3)
    return np.ascontiguousarray(h.reshape(ngrp * P, subs * BL))


def _make_in_maps(x0, eps, mm_dtype=MM_DTYPE):
    et = _make_e(x0, eps)
    m0, m1 = _toeplitz()
    if mm_dtype.startswith("fp16p"):
        subs = int(mm_dtype[5:]) if len(mm_dtype) > 5 else 2
        m0h = m0.astype(np.float16)
        m1h = m1.astype(np.float16)
        return [
            {
                "einp": _pack_input_fp16(et[:, c * BL:(c + 1) * BL], subs),
                "m0": m0h,
                "m1": m1h,
            }
            for c in range(NCORES)
        ]
    if mm_dtype == "bf16x2p":
        m0h, m0l = _split_bf16(m0)
        m1h, m1l = _split_bf16(m1)
        return [
            {
                "einp": _pack_input(et[:, c * BL:(c + 1) * BL]),
                "m0h": m0h, "m0l": m0l, "m1h": m1h, "m1l": m1l,
            }
            for c in range(NCORES)
        ]
    if mm_dtype == "f32":
        return [
            {
                "ein": np.ascontiguousarray(et[:, c * BL:(c + 1) * BL]),
                "m0": m0,
                "m1": m1,
            }
            for c in range(NCORES)
        ]
    m0h, m0l = _split_bf16(m0)
    m1h, m1l = _split_bf16(m1)
    maps = []
    for c in range(NCORES):
        shard = et[:, c * BL:(c + 1) * BL]
        hi, lo = _split_bf16(shard)
        maps.append(
            {
                "ein_hi": np.ascontiguousarray(hi),
                "ein_lo": np.ascontiguousarray(lo),
                "m0h": m0h, "m0l": m0l, "m1h": m1h, "m1l": m1l,
            }
        )
    return maps


def _run(in_maps, mm_dtype=MM_DTYPE, **kwargs):
    from concourse.bass_utils import run_bass_kernel_spmd

    nc = _get_nc(mm_dtype)
    return run_bass_kernel_spmd(
        nc, in_maps, core_ids=list(range(NCORES)), **kwargs
    )


def _gather(res, mm_dtype=MM_DTYPE):
    out = np.empty((B, T), np.float32)
    for c in range(NCORES):
        if mm_dtype.startswith("fp16p"):
            subs = int(mm_dtype[5:]) if len(mm_dtype) > 5 else 2
            out[c * BL:(c + 1) * BL, :] = _unpack_output(
                res.results[c]["outp"], subs
            )
        elif mm_dtype == "bf16x2p":
            out[c * BL:(c + 1) * BL, :] = _unpack_output(res.results[c]["outp"])
        else:
            out[c * BL:(c + 1) * BL, :] = res.results[c]["out"].T
    return out


def kernel(x0, eps):
    res = _run(_make_in_maps(x0, eps))
    return _gather(res)


# revision 21
# speedup vs baseline: 1.1427x; 1.1427x over previous
"""AR(1) sequence generation kernel for Trainium2 (8 NeuronCores).

x_t = COEFF * x_{t-1} + STD * eps_t  with x_0 given; output [B, T] f32.

Algorithm: the recurrence is a causal convolution x = conv(E, k) with
k[d] = STD * COEFF**d, where E[:,0] = x0/STD and E[:,1:] = eps.  Since
COEFF**128 ~ 4e-13 (far below f32 ulp), a 128-wide output time-block m
only depends on time-blocks m and m-1:

    X[:, m*128:(m+1)*128] = E_m @ M0 + E_{m-1} @ M1

with constant 128x128 Toeplitz matrices M0[j,i] = k[i-j] (i>=j) and
M1[j,i] = k[128+i-j].  This removes the serial scan entirely.

Precision/speed: operands are split on the host into bf16 hi + bf16 lo
(same DMA bytes as f32).  Each logical f32 matmul becomes three bf16
matmuls accumulated in f32 PSUM: hi@hi + hi@lo + lo@hi, giving ~1e-5
relative error (vs ~3e-4 for TF32) at full bf16 TensorEngine rate.

Sharding: pure data parallel over batch (8192 -> 8 x 1024).  Compute
uses a transposed layout (time on partitions, batch on the free dim) so
the constant matrices are the stationary operand and the moving operand
streams 512 batch columns per matmul; the host transposes input/output.
"""

import numpy as np

COEFF = 0.8
STD = 0.1
B, T = 8192, 4096
NCORES = 8
BL = B // NCORES  # 1024 batch rows per core
P = 128           # partition dim / time block
NBLK = T // P     # 32 time blocks
NCHUNK = BL // 512  # 2 batch chunks of 512

# "bf16x2p" (hi/lo split + packed DMA layout), "bf16x2" (hi/lo split),
# or "f32" (exact, 4x slower PE)
MM_DTYPE = "fp16p"
NSB = NBLK // 2  # superblocks of 2 time blocks (packed layout)

_CACHE = {}


def _toeplitz():
    j = np.arange(P, dtype=np.float64)[:, None]
    i = np.arange(P, dtype=np.float64)[None, :]
    m0 = np.where(i >= j, STD * COEFF ** (i - j), 0.0).astype(np.float32)
    m1 = (STD * COEFF ** (P + i - j)).astype(np.float32)
    return m0, m1


def _split_bf16(a):
    import ml_dtypes

    bf16 = ml_dtypes.bfloat16
    hi = a.astype(bf16)
    lo = (a - hi.astype(np.float32)).astype(bf16)
    return hi, lo


def _build_nc_f32():
    import concourse.bacc as bacc
    import concourse.tile as tile
    import concourse.mybir as mybir

    f32 = mybir.dt.float32
    nc = bacc.Bacc("TRN2", target_bir_lowering=False, debug=False)
    ein = nc.dram_tensor("ein", [T, BL], f32, kind="ExternalInput")
    m0 = nc.dram_tensor("m0", [P, P], f32, kind="ExternalInput")
    m1 = nc.dram_tensor("m1", [P, P], f32, kind="ExternalInput")
    out = nc.dram_tensor("out", [T, BL], f32, kind="ExternalOutput")

    with tile.TileContext(nc) as tc:
        with (
            tc.tile_pool(name="consts", bufs=1) as consts,
            tc.tile_pool(name="einp", bufs=4) as einp,
            tc.tile_pool(name="stage", bufs=3) as stage,
            tc.tile_pool(name="psum", bufs=4, space="PSUM") as psump,
        ):
            m0t = consts.tile([P, P], f32, name="m0t")
            nc.sync.dma_start(m0t[:], m0[:, :])
            m1t = consts.tile([P, P], f32, name="m1t")
            nc.sync.dma_start(m1t[:], m1[:, :])

            einv = ein[:, :].rearrange("(n p) b -> n p b", p=P)
            outv = out[:, :].rearrange("(n p) b -> n p b", p=P)

            prev = None
            for m in range(NBLK):
                et = einp.tile([P, BL], f32, name="et")
                nc.sync.dma_start(et[:], einv[m])
                st = stage.tile([P, BL], f32, name="st")
                for c in range(NCHUNK):
                    ps = psump.tile([P, 512], f32, name="ps")
                    sl = slice(c * 512, (c + 1) * 512)
                    nc.tensor.matmul(
                        ps[:], m0t[:], et[:, sl],
                        start=True, stop=(prev is None),
                    )
                    if prev is not None:
                        nc.tensor.matmul(
                            ps[:], m1t[:], prev[:, sl],
                            start=False, stop=True,
                        )
                    nc.vector.tensor_copy(out=st[:, sl], in_=ps[:])
                nc.sync.dma_start(outv[m], st[:])
                prev = et
    nc.compile()
    return nc


def _build_nc_bf16x2():
    import concourse.bacc as bacc
    import concourse.tile as tile
    import concourse.mybir as mybir

    f32 = mybir.dt.float32
    bf16 = mybir.dt.bfloat16
    nc = bacc.Bacc("TRN2", target_bir_lowering=False, debug=False)
    ein_hi = nc.dram_tensor("ein_hi", [T, BL], bf16, kind="ExternalInput")
    ein_lo = nc.dram_tensor("ein_lo", [T, BL], bf16, kind="ExternalInput")
    consts_in = {}
    for name in ("m0h", "m0l", "m1h", "m1l"):
        consts_in[name] = nc.dram_tensor(name, [P, P], bf16, kind="ExternalInput")
    out = nc.dram_tensor("out", [T, BL], f32, kind="ExternalOutput")

    with tile.TileContext(nc) as tc:
        with (
            tc.tile_pool(name="consts", bufs=1) as consts,
            tc.tile_pool(name="einp", bufs=8) as einp,
            tc.tile_pool(name="stage", bufs=3) as stage,
            tc.tile_pool(name="psum", bufs=4, space="PSUM") as psump,
        ):
            ct = {}
            for name in ("m0h", "m0l", "m1h", "m1l"):
                ct[name] = consts.tile([P, P], bf16, name=name + "t")
                nc.sync.dma_start(ct[name][:], consts_in[name][:, :])

            ehv = ein_hi[:, :].rearrange("(n p) b -> n p b", p=P)
            elv = ein_lo[:, :].rearrange("(n p) b -> n p b", p=P)
            outv = out[:, :].rearrange("(n p) b -> n p b", p=P)

            prev = None
            for m in range(NBLK):
                eh = einp.tile([P, BL], bf16, name="eh", tag="eh")
                nc.sync.dma_start(eh[:], ehv[m])
                el = einp.tile([P, BL], bf16, name="el", tag="el")
                nc.sync.dma_start(el[:], elv[m])
                st = stage.tile([P, BL], f32, name="st")
                for c in range(NCHUNK):
                    sl = slice(c * 512, (c + 1) * 512)
                    ps = psump.tile([P, 512], f32, name="ps")
                    # (weights, moving) products accumulated in PSUM;
                    # ordered so equal stationary operands are adjacent.
                    prods = [
                        (ct["m0h"], eh),
                        (ct["m0h"], el),
                        (ct["m0l"], eh),
                    ]
                    if prev is not None:
                        ph, pl = prev
                        prods += [
                            (ct["m1h"], ph),
                            (ct["m1h"], pl),
                            (ct["m1l"], ph),
                        ]
                    last = len(prods) - 1
                    for idx, (w, mv) in enumerate(prods):
                        nc.tensor.matmul(
                            ps[:], w[:], mv[:, sl],
                            start=(idx == 0), stop=(idx == last),
                        )
                    nc.vector.tensor_copy(out=st[:, sl], in_=ps[:])
                nc.sync.dma_start(outv[m], st[:])
                prev = (eh, el)
    nc.compile()
    return nc


def _build_nc_bf16x2p():
    """Packed layout: input DRAM tensor [NSB*128, 4096] bf16 where the
    row (sb*128 + p) holds, contiguously: [hi(sub0) | lo(sub0) | hi(sub1)
    | lo(sub1)] each 1024 wide, with sub-block rows interleaved so every
    partition line is one contiguous 8KB DMA descriptor.  Output DRAM is
    [NSB*128, 2048] f32: row (sb*128+p) = [x^T row 256sb+p | row
    256sb+128+p] (8KB lines); the host unpacks."""
    import concourse.bacc as bacc
    import concourse.tile as tile
    import concourse.mybir as mybir

    f32 = mybir.dt.float32
    bf16 = mybir.dt.bfloat16
    nc = bacc.Bacc("TRN2", target_bir_lowering=False, debug=False)
    einp = nc.dram_tensor("einp", [NSB * P, 4096], bf16, kind="ExternalInput")
    consts_in = {}
    for name in ("m0h", "m0l", "m1h", "m1l"):
        consts_in[name] = nc.dram_tensor(name, [P, P], bf16, kind="ExternalInput")
    outp = nc.dram_tensor("outp", [NSB * P, 2048], f32, kind="ExternalOutput")

    with tile.TileContext(nc) as tc:
        with (
            tc.tile_pool(name="consts", bufs=1) as consts,
            tc.tile_pool(name="ein", bufs=3) as ein_pool,
            tc.tile_pool(name="stage", bufs=3) as stage,
            tc.tile_pool(name="psum", bufs=8, space="PSUM") as psump,
        ):
            ct = {}
            for name in ("m0h", "m0l", "m1h", "m1l"):
                ct[name] = consts.tile([P, P], bf16, name=name + "t")
                nc.gpsimd.dma_start(ct[name][:], consts_in[name][:, :])

            # [NSB, p, sub, hilo, 1024]
            einv = einp[:, :].rearrange(
                "(n p) (s h b) -> n p s h b", p=P, s=2, h=2
            )
            outv = outp[:, :].rearrange("(n p) (s b) -> n p s b", p=P, s=2)

            prev = None  # (tile, sub) holding time block m-1
            for sb in range(NSB):
                et = ein_pool.tile([P, 2, 2, 1024], bf16, name="et")
                nc.sync.dma_start(et[:], einv[sb])
                st = stage.tile([P, 2, 1024], f32, name="st")
                for sub in range(2):
                    eh = et[:, sub, 0, :]
                    el = et[:, sub, 1, :]
                    for c in range(NCHUNK):
                        sl = slice(c * 512, (c + 1) * 512)
                        ps = psump.tile([P, 512], f32, name="ps")
                        prods = [
                            (ct["m0h"], eh),
                            (ct["m0h"], el),
                            (ct["m0l"], eh),
                        ]
                        if prev is not None:
                            pt, psub = prev
                            prods += [
                                (ct["m1h"], pt[:, psub, 0, :]),
                                (ct["m1h"], pt[:, psub, 1, :]),
                                (ct["m1l"], pt[:, psub, 0, :]),
                            ]
                        last = len(prods) - 1
                        for idx, (w, mv) in enumerate(prods):
                            nc.tensor.matmul(
                                ps[:], w[:], mv[:, sl],
                                start=(idx == 0), stop=(idx == last),
                            )
                        nc.vector.tensor_copy(out=st[:, sub, sl], in_=ps[:])
                    prev = (et, sub)
                nc.scalar.dma_start(outv[sb], st[:])
    nc.compile()
    return nc


def _build_nc_fp16p(subs=2, ein_bufs=6, stage_bufs=4):
    """Pure fp16 operands (rel err ~3e-4), packed DMA layout.

    Input DRAM [(NBLK/subs)*P, subs*1024] fp16: row (g*P+p) holds the
    `subs` E^T rows {subs*128*g + 128*s + p} concatenated, so every
    partition line is one contiguous (2KB*subs) DMA descriptor.  Output
    DRAM is packed the same way in f32 (4KB*subs lines); the host
    unpacks.  Two matmul products per output chunk."""
    import concourse.bacc as bacc
    import concourse.tile as tile
    import concourse.mybir as mybir

    ngrp = NBLK // subs
    f32 = mybir.dt.float32
    f16 = mybir.dt.float16
    nc = bacc.Bacc("TRN2", target_bir_lowering=False, debug=False)
    einp = nc.dram_tensor("einp", [ngrp * P, subs * BL], f16, kind="ExternalInput")
    m0_in = nc.dram_tensor("m0", [P, P], f16, kind="ExternalInput")
    m1_in = nc.dram_tensor("m1", [P, P], f16, kind="ExternalInput")
    outp = nc.dram_tensor("outp", [ngrp * P, subs * BL], f32, kind="ExternalOutput")

    with tile.TileContext(nc) as tc:
        with (
            tc.tile_pool(name="consts", bufs=1) as consts,
            tc.tile_pool(name="ein", bufs=ein_bufs) as ein_pool,
            tc.tile_pool(name="stage", bufs=stage_bufs) as stage,
            tc.tile_pool(name="psum", bufs=8, space="PSUM") as psump,
        ):
            m0t = consts.tile([P, P], f16, name="m0t")
            nc.sync.dma_start(m0t[:], m0_in[:, :])
            m1t = consts.tile([P, P], f16, name="m1t")
            nc.sync.dma_start(m1t[:], m1_in[:, :])

            einv = einp[:, :].rearrange("(n p) (s b) -> n p s b", p=P, s=subs)
            outv = outp[:, :].rearrange("(n p) (s b) -> n p s b", p=P, s=subs)

            prev = None  # (tile, sub) holding time block m-1
            for g in range(ngrp):
                et = ein_pool.tile([P, subs, BL], f16, name="et")
                nc.sync.dma_start(et[:], einv[g])
                st = stage.tile([P, subs, BL], f32, name="st")
                for sub in range(subs):
                    for c in range(NCHUNK):
                        sl = slice(c * 512, (c + 1) * 512)
                        ps = psump.tile([P, 512], f32, name="ps")
                        if prev is None:
                            nc.tensor.matmul(
                                ps[:], m0t[:], et[:, sub, sl],
                                start=True, stop=True,
                            )
                        else:
                            pt, psub = prev
                            nc.tensor.matmul(
                                ps[:], m0t[:], et[:, sub, sl],
                                start=True, stop=False,
                            )
                            nc.tensor.matmul(
                                ps[:], m1t[:], pt[:, psub, sl],
                                start=False, stop=True,
                            )
                        if (sub + c) % 2 == 0:
                            nc.vector.tensor_copy(out=st[:, sub, sl], in_=ps[:])
                        else:
                            nc.scalar.copy(out=st[:, sub, sl], in_=ps[:])
                    prev = (et, sub)
                nc.scalar.dma_start(outv[g], st[:])
    nc.compile()
    return nc


def _get_nc(mm_dtype=MM_DTYPE):
    if mm_dtype not in _CACHE:
        if mm_dtype.startswith("fp16p"):
            subs = int(mm_dtype[5:]) if len(mm_dtype) > 5 else 2
            kw = {"ein_bufs": 3, "stage_bufs": 3} if subs >= 8 else (
                {"ein_bufs": 8, "stage_bufs": 5} if subs == 4 else {})
            _CACHE[mm_dtype] = _build_nc_fp16p(subs=subs, **kw)
        elif mm_dtype == "bf16x2p":
            _CACHE[mm_dtype] = _build_nc_bf16x2p()
        elif mm_dtype == "bf16x2":
            _CACHE[mm_dtype] = _build_nc_bf16x2()
        elif mm_dtype == "f32":
            _CACHE[mm_dtype] = _build_nc_f32()
        else:
            raise ValueError(mm_dtype)
    return _CACHE[mm_dtype]


def _make_e(x0, eps):
    e = np.empty((B, T), np.float32)
    e[:, 0] = (x0[:, 0].astype(np.float64) / STD).astype(np.float32)
    e[:, 1:] = eps
    return np.ascontiguousarray(e.T)  # [T, B]


def _pack_input(shard):
    """shard: [T, BL] f32 (x^T for one core) -> [NSB*P, 4096] bf16 packed."""
    hi, lo = _split_bf16(shard)
    # [NSB, sub, P, BL] -> [NSB, P, sub, BL]
    hi4 = hi.reshape(NSB, 2, P, BL).transpose(0, 2, 1, 3)
    lo4 = lo.reshape(NSB, 2, P, BL).transpose(0, 2, 1, 3)
    packed = np.empty((NSB, P, 2, 2, BL), hi.dtype)
    packed[:, :, :, 0, :] = hi4
    packed[:, :, :, 1, :] = lo4
    return np.ascontiguousarray(packed.reshape(NSB * P, 4 * BL))


def _unpack_output(arr, subs=2):
    """arr: [(NBLK/subs)*P, subs*BL] f32 -> [BL, T] (batch-major shard)."""
    ngrp = NBLK // subs
    a = arr.reshape(ngrp, P, subs, BL).transpose(0, 2, 1, 3).reshape(T, BL)
    return a.T


def _pack_input_fp16(shard, subs=2):
    """shard: [T, BL] f32 -> [(NBLK/subs)*P, subs*BL] fp16 packed."""
    ngrp = NBLK // subs
    h = shard.astype(np.float16).reshape(ngrp, subs, P, BL).transpose(0, 2, 1, 3)
    return np.ascontiguousarray(h.reshape(ngrp * P, subs * BL))


def _make_in_maps(x0, eps, mm_dtype=MM_DTYPE):
    et = _make_e(x0, eps)
    m0, m1 = _toeplitz()
    if mm_dtype.startswith("fp16p"):
        subs = int(mm_dtype[5:]) if len(mm_dtype) > 5 else 2
        m0h = m0.astype(np.float16)
        m1h = m1.astype(np.float16)
        return [
            {
                "einp": _pack_input_fp16(et[:, c * BL:(c + 1) * BL], subs),
                "m0": m0h,
                "m1": m1h,
            }
            for c in range(NCORES)
        ]
    if mm_dtype == "bf16x2p":
        m0h, m0l = _split_bf16(m0)
        m1h, m1l = _split_bf16(m1)
        return [
            {
                "einp": _pack_input(et[:, c * BL:(c + 1) * BL]),
                "m0h": m0h, "m0l": m0l, "m1h": m1h, "m1l": m1l,
            }
            for c in range(NCORES)
        ]
    if mm_dtype == "f32":
        return [
            {
                "ein": np.ascontiguousarray(et[:, c * BL:(c + 1) * BL]),
                "m0": m0,
                "m1": m1,
            }
            for c in range(NCORES)
        ]
    m0h, m0l = _split_bf16(m0)
    m1h, m1l = _split_bf16(m1)
    maps = []
    for c in range(NCORES):
        shard = et[:, c * BL:(c + 1) * BL]
        hi, lo = _split_bf16(shard)
        maps.append(
            {
                "ein_hi": np.ascontiguousarray(hi),
                "ein_lo": np.ascontiguousarray(lo),
                "m0h": m0h, "m0l": m0l, "m1h": m1h, "m1l": m1l,
            }
        )
    return maps


def _run(in_maps, mm_dtype=MM_DTYPE, **kwargs):
    from concourse.bass_utils import run_bass_kernel_spmd

    nc = _get_nc(mm_dtype)
    return run_bass_kernel_spmd(
        nc, in_maps, core_ids=list(range(NCORES)), **kwargs
    )


def _gather(res, mm_dtype=MM_DTYPE):
    out = np.empty((B, T), np.float32)
    for c in range(NCORES):
        if mm_dtype.startswith("fp16p"):
            subs = int(mm_dtype[5:]) if len(mm_dtype) > 5 else 2
            out[c * BL:(c + 1) * BL, :] = _unpack_output(
                res.results[c]["outp"], subs
            )
        elif mm_dtype == "bf16x2p":
            out[c * BL:(c + 1) * BL, :] = _unpack_output(res.results[c]["outp"])
        else:
            out[c * BL:(c + 1) * BL, :] = res.results[c]["out"].T
    return out


def kernel(x0, eps):
    res = _run(_make_in_maps(x0, eps))
    return _gather(res)
